# revision 18
# baseline (speedup 1.0000x reference)
"""Fused causal multi-head attention block on 8 Trainium2 NeuronCores.

Problem (GPT-2 style attention, B=2, S=2048, D=1024, H=16, hd=64):
    qkv = x @ w_attn + b_attn ; split q,k,v ; per-head causal softmax(q k^T / 8) v
    out = attn_out @ w_proj + b_proj

Sharding: data parallel on batch (2) x tensor parallel on heads (4 groups of 4
heads). Core c -> batch c//4, head group c%4. Each core computes a partial
[S, D] output (its heads' slice of w_proj rows); host sums the 4 partials per
batch and adds b_proj.

Per-core kernel layout tricks:
- scores are computed TRANSPOSED (scoresT[key, query]) so the softmax
  denominator falls out of the attn@v matmul by appending a ones-column to v:
  [v | 1]^T @ exp(scoresT) yields the unnormalized output and the per-query
  denominator in one PSUM accumulation.
- matmul inputs are fp16 (full PE rate + fast weight loads); all accumulation
  is fp32 in PSUM. exp(s/8) is in [0, ~13], well inside fp16 range.
- causal masking: fully-masked blocks are skipped via restricted matmul
  widths; diagonal blocks get their exp output multiplied by a 0/1 triangle
  on the (otherwise idle) GpSimd engine, keeping Vector free for psum copies.
- all DRAM tensors are host-relayouted to partition-major contiguous form so
  every DMA issue is a cheap 128-line descriptor; x is loaded per 512-query
  chunk (chunk 0 first) and weights stream in parallel on the Scalar HWDGE
  ring, so real QKV work starts ~8us in with no junk warmup needed.
- emission is chunk-pipelined (QKV chunk c, attention chunk c, projection
  chunk c) so the PE always has dense matmul work while ScalarE runs exp;
  each pair's normalization matmuls are deferred past the next pair's score
  matmuls so the in-order PE never waits on Vector/GpSimd latency.
"""

import sys

sys.path.insert(0, "/opt/trn_rl_repo")

import numpy as np

import concourse.bass as bass
import concourse.mybir as mybir
import concourse.tile as tile
from concourse import bacc
from concourse.bass_utils import run_bass_kernel_spmd

F32 = mybir.dt.float32
F16 = mybir.dt.float16
AFT = mybir.ActivationFunctionType

B, S, D, H, HD = 2, 2048, 1024, 16, 64
NCORES = 8
HPC = 4            # heads per core
CH = HPC * HD      # 256 channels per core
VW = HD + 1        # v width incl. ones column
P = 128
KT = D // P        # 8 contraction tiles over D
SQ = 512           # query/N chunk
NSQ = S // SQ      # 4
NST = S // P       # 16 seq tiles
SCALE = 1.0 / np.sqrt(HD)


def emit_kernel(nc, tc, ap):
    """Emit the per-core program. `ap` is a dict of DRAM APs."""
    with (
        tc.tile_pool(name="const", bufs=1) as cp,
        tc.tile_pool(name="xw", bufs=1) as xw,
        tc.tile_pool(name="act", bufs=1) as acts,
        tc.tile_pool(name="ex", bufs=20) as exp_pool,
        tc.tile_pool(name="dh", bufs=4) as dh_pool,
        tc.tile_pool(name="rc", bufs=2) as rc_pool,
        tc.tile_pool(name="osb", bufs=3) as osb,
        tc.tile_pool(name="psA", bufs=2, space="PSUM") as psA,
        tc.tile_pool(name="psB", bufs=2, space="PSUM") as psB,
        tc.tile_pool(name="psC", bufs=2, space="PSUM") as psC,
    ):
        # ---- input DMAs. Two parallel HWDGE rings: x chunks + small consts
        # on Sync, weights on Scalar. Chunk-0 x and wq are split in halves so
        # the first QKV matmuls can start after ~0.75MB instead of ~1.5MB.
        xts = xw.tile([P, NSQ, KT, SQ], F16, name="xts", tag="xts")
        half = KT // 2
        nc.sync.dma_start(xts[:, 0, 0:half], ap["xln"][:, 0, 0:half])
        nc.sync.dma_start(xts[:, 0, half:KT], ap["xln"][:, 0, half:KT])
        tri = cp.tile([P, P], F16, name="tri", tag="tri")
        nc.sync.dma_start(tri, ap["tri"])
        bqk = cp.tile([P, 4], F32, name="bqk", tag="bqk")
        nc.sync.dma_start(bqk, ap["bqk"])
        bv = cp.tile([1, HPC * VW], F16, name="bv", tag="bv")
        nc.sync.dma_start(bv, ap["bv"])
        ones1 = cp.tile([1, P], F16, name="ones1", tag="ones1")
        nc.sync.dma_start(ones1, ap["ones1"])
        for c in range(1, NSQ):
            nc.sync.dma_start(xts[:, c], ap["xln"][:, c])

        wq = xw.tile([P, KT, CH], F16, name="wq", tag="wq")
        wq_ap = ap["wq"].rearrange("p (k c) -> p k c", k=KT)
        nc.scalar.dma_start(wq[:, 0:half], wq_ap[:, 0:half])
        nc.scalar.dma_start(wq[:, half:KT], wq_ap[:, half:KT])
        wk = xw.tile([P, KT, CH], F16, name="wk", tag="wk")
        nc.scalar.dma_start(wk, ap["wk"].rearrange("p (k c) -> p k c", k=KT))
        wv = xw.tile([P, KT, HPC * VW], F16, name="wv", tag="wv")
        nc.scalar.dma_start(wv, ap["wv"].rearrange("p (k c) -> p k c", k=KT))
        wp = xw.tile([P, 2, D], F16, name="wp", tag="wp")
        nc.scalar.dma_start(wp, ap["wp"].rearrange("p (k c) -> p k c", k=2))

        wq_t = [wq[:, k, :] for k in range(KT)]
        wk_t = [wk[:, k, :] for k in range(KT)]
        wv_t = [wv[:, k, :] for k in range(KT)]
        wp_t = [wp[:, k, :] for k in range(2)]

        # ---- activations living across phases ----
        qT = [acts.tile([P, S], F16, name=f"qT{i}", tag=f"qT{i}") for i in range(2)]
        kTt = [acts.tile([P, S], F16, name=f"kT{i}", tag=f"kT{i}") for i in range(2)]
        vv = acts.tile([P, NST, HPC * VW], F16, name="vv", tag="vv")
        outT = [acts.tile([P, S], F16, name=f"oT{i}", tag=f"oT{i}") for i in range(2)]

        def qkv_qk_group(c, dst, wt, bcol, i):
            ps = psA.tile([P, SQ], F32, name="ps", tag="ps")
            for k in range(KT):
                nc.tensor.matmul(
                    ps,
                    wt[k][:, i * P:(i + 1) * P],
                    xts[:, c, k, :],
                    start=(k == 0),
                    stop=(k == KT - 1),
                )
            with nc.allow_low_precision(reason="fp16 matmul inputs"):
                nc.vector.tensor_scalar_add(
                    dst[i][:, c * SQ:(c + 1) * SQ], ps, bqk[:, bcol + i:bcol + i + 1],
                )

        def qkv_v_group(st):
            # v rows (natural layout + interleaved ones cols)
            ps = psA.tile([P, SQ], F32, name="psv", tag="ps")
            psv = ps[:, 0:HPC * VW]
            for k in range(KT):
                nc.tensor.matmul(
                    psv,
                    xts[:, st // 4, k, (st % 4) * P:(st % 4 + 1) * P],
                    wv_t[k],
                    start=(k == 0),
                    stop=False,
                )
            # += ones_col(seq) x (bv | interleaved 1.0): v-bias + ones col
            nc.tensor.matmul(psv, ones1, bv, start=False, stop=True)
            with nc.allow_low_precision(reason="fp16 matmul inputs"):
                nc.vector.tensor_copy(vv[:, st, :], psv)

        def qkv_groups(c):
            for dst, wt, bcol in ((qT, wq_t, 0), (kTt, wk_t, 2)):
                for i in range(2):
                    yield lambda dst=dst, wt=wt, bcol=bcol, i=i: \
                        qkv_qk_group(c, dst, wt, bcol, i)
            for st in range(4 * c, 4 * c + 4):
                yield lambda st=st: qkv_v_group(st)

        def make_scores(i, c):
            """Score matmul + exp emission closures for pair (c, i), one per
            key tile. Both heads' scores for a key tile land in one 2-bank
            PSUM tile so a single exp instruction covers them. Diagonal key
            tiles go FIRST so their exp->mask chain (mask on the idle GpSimd)
            completes long before attnv consumes them. The closures are
            interleaved into the PREVIOUS pair's attnv phase so ScalarE's exp
            stream never starves at pair boundaries."""
            nkt = 4 * (c + 1)
            kts = list(range(4 * c, nkt)) + list(range(0, 4 * c))
            exs = []

            def emit_kt(kt):
                colo = max(0, kt * P - c * SQ)
                diag = colo > 0 or kt * P == c * SQ
                sc2 = psC.tile([P, 2, SQ], F32, name="sc2", tag="sc")
                for j in range(2):
                    ro = j * 64
                    nc.tensor.matmul(
                        sc2[:, j, colo:SQ],
                        kTt[i][ro:ro + 64, kt * P:(kt + 1) * P],
                        qT[i][ro:ro + 64, c * SQ + colo:(c + 1) * SQ],
                        start=True,
                        stop=True,
                    )
                ex2 = exp_pool.tile([P, 2, SQ], F16, name="ex2", tag="ex")
                nc.scalar.activation(
                    ex2[:, :, colo:SQ], sc2[:, :, colo:SQ], AFT.Exp, scale=SCALE,
                )
                if diag:
                    nc.gpsimd.tensor_mul(
                        ex2[:, :, colo:colo + P],
                        ex2[:, :, colo:colo + P],
                        tri[:, None, :].broadcast_to([P, 2, P]),
                    )
                exs.append((ex2, kt, colo))

            return [(lambda kt=kt: emit_kt(kt)) for kt in kts], exs

        def do_attnv(i, c, exs, fillers, next_scores, dn_first=False):
            """attnv accumulation for pair (c, i), interleaved with the next
            pair's score/exp emissions (to keep ScalarE saturated) and dense
            PE fillers (qkv/proj work)."""
            nkt = 4 * (c + 1)
            fillers = list(fillers)
            next_scores = list(next_scores)
            nf, ns = len(fillers), len(next_scores)
            accs = [psB.tile([VW, SQ], F32, name="acc", tag="acc")
                    for _ in range(2)]
            # pre-work before the first attnv matmul: covers the PSUM
            # acc-slot WAR on the previous pair's psum->sbuf copies and
            # jump-starts the next pair's exp stream
            for pre in range(2):
                if fillers:
                    fillers.pop(0)()
                if next_scores:
                    next_scores.pop(0)()
            for idx, (ex2, kt, colo) in enumerate(exs):
                for j in range(2):
                    h = 2 * i + j
                    nc.tensor.matmul(
                        accs[j][:, colo:SQ],
                        vv[:, kt, h * VW:(h + 1) * VW],
                        ex2[:, j, colo:SQ],
                        start=(idx == 0),
                        stop=(idx == nkt - 1),
                    )
                while next_scores and \
                        (ns - len(next_scores)) < ns * (idx + 1) // nkt:
                    next_scores.pop(0)()
                while fillers and len(fillers) > nf * (nkt - 1 - idx) // nkt:
                    fillers.pop(0)()
            # psum->sbuf copies; outT first so the next pair's attnv can
            # reuse the acc psum slots ASAP. At the tail (dn_first) the norm
            # chain is the critical path: dns go first on Vector and the outT
            # copies move to the now-idle ScalarE so Vector's serial chain is
            # just dn -> reciprocal -> mul.
            dns = []
            if dn_first:
                for j in range(2):
                    dn = dh_pool.tile([1, SQ], F16, name="dn", tag="dn")
                    with nc.allow_low_precision(reason="fp16 matmul inputs"):
                        nc.vector.tensor_copy(dn, accs[j][64:65, :])
                    dns.append(dn)
            for j in range(2):
                with nc.allow_low_precision(reason="fp16 matmul inputs"):
                    if dn_first:
                        nc.scalar.activation(
                            outT[i][j * 64:j * 64 + 64, c * SQ:(c + 1) * SQ],
                            accs[j][0:64, :], AFT.Copy,
                        )
                    else:
                        nc.vector.tensor_copy(
                            outT[i][j * 64:j * 64 + 64, c * SQ:(c + 1) * SQ],
                            accs[j][0:64, :],
                        )
            if not dn_first:
                for j in range(2):
                    dn = dh_pool.tile([1, SQ], F16, name="dn", tag="dn")
                    with nc.allow_low_precision(reason="fp16 matmul inputs"):
                        nc.vector.tensor_copy(dn, accs[j][64:65, :])
                    dns.append(dn)
            return dns

        def norm_pair(c, i, dns, on_vector=False, db_pool=None):
            # outT *= 1/denominator: broadcast denoms via K=1 matmuls, one
            # 128-lane fast reciprocal, one fp16 multiply (on GpSimd
            # mid-kernel; on Vector at the tail where latency matters)
            db = (db_pool or psA).tile([P, SQ], F32, name="db", tag="acc" if db_pool else "ps")
            nc.tensor.matmul(
                db[0:64, :], ones1[:, 0:64], dns[0],
                start=True, stop=True,
            )
            nc.tensor.matmul(
                db[64:P, :], ones1[:, 0:64], dns[1],
                start=True, stop=True,
            )
            rc32 = rc_pool.tile([P, SQ], F32, name="rc32", tag="rc32")
            nc.vector.reciprocal_approx_fast(rc32, db)
            eng = nc.vector if on_vector else nc.gpsimd
            with nc.allow_low_precision(reason="fp16 matmul inputs"):
                eng.tensor_mul(
                    outT[i][:, c * SQ:(c + 1) * SQ],
                    outT[i][:, c * SQ:(c + 1) * SQ],
                    rc32,
                )

        def proj_mtile(m, split_dma=False):
            ob = osb.tile([P, D], F16, name="ob", tag="ob")
            for nch in range(2):
                ps = psA.tile([P, SQ], F32, name="ps", tag="ps")
                for kk in range(2):
                    nc.tensor.matmul(
                        ps,
                        outT[kk][:, m * P:(m + 1) * P],
                        wp_t[kk][:, nch * SQ:(nch + 1) * SQ],
                        start=(kk == 0),
                        stop=(kk == 1),
                    )
                with nc.allow_low_precision(reason="partial sums; host sums fp32"):
                    nc.vector.tensor_copy(ob[:, nch * SQ:(nch + 1) * SQ], ps)
                if split_dma:
                    nc.sync.dma_start(
                        ap["out"][m * P:(m + 1) * P, nch * SQ:(nch + 1) * SQ],
                        ob[:, nch * SQ:(nch + 1) * SQ],
                    )
            if not split_dma:
                nc.sync.dma_start(ap["out"][m * P:(m + 1) * P, :], ob)

        pf_ps = {}

        def proj_kk0(m):
            # kk=0 contraction half of a tail projection tile, parked in
            # psC (free once the last exps drain) until norm(3,1) releases
            # outT[1] for the kk=1 half
            ps2 = psC.tile([P, 2, SQ], F32, name="pf", tag="sc")
            pf_ps[m] = [ps2[:, 0, :], ps2[:, 1, :]]
            for nch in range(2):
                nc.tensor.matmul(
                    pf_ps[m][nch],
                    outT[0][:, m * P:(m + 1) * P],
                    wp_t[0][:, nch * SQ:(nch + 1) * SQ],
                    start=True,
                    stop=False,
                )

        # ---- chunk-pipelined main body ----
        # chunk 0 QKV runs as soon as chunk-0 x + weights stream in; the
        # PE's HAM cold-start window is spent on this real work.
        for g in qkv_groups(0):
            g()
        # Pair p's scores/exps are interleaved into pair p-1's attnv phase
        # so both the PE and ScalarE stay saturated; qkv chunk c+1 and the
        # ready projection tiles serve as dense PE filler inside the
        # exp-paced attnv windows. Each pair's norm is deferred two slots
        # into the NEXT pair's filler stream (past the Vector dn-copies).
        pairs = [(c, i) for c in range(NSQ) for i in range(2)]
        cl0, cur_exs = make_scores(0, 0)
        for cl in cl0:
            cl()
        pending_norm = None
        for pidx, (c, i) in enumerate(pairs):
            last = pidx == len(pairs) - 1
            if not last:
                nxt_c, nxt_i = pairs[pidx + 1]
                nxt_cl, nxt_exs = make_scores(nxt_i, nxt_c)
            else:
                nxt_cl, nxt_exs = [], None
            fillers = []
            if c + 1 < NSQ:
                nxt_qkv = list(qkv_groups(c + 1))
                fillers += nxt_qkv[4 * i:4 * i + 4]
            if c == NSQ - 2 and i == 1:
                fillers += [lambda: proj_mtile(0), lambda: proj_mtile(1)]
            if c == NSQ - 1:
                ms = range(2, 8) if i == 0 else range(8, 12)
                fillers += [(lambda m=m: proj_mtile(m)) for m in ms]
            if c == NSQ - 1 and i == 1:
                # prefetch the kk=0 contraction half of the first two tail
                # projection tiles (outT[0] chunk 3 is normalized mid-phase)
                # into psC, which is free once the last exps drain
                fillers += [(lambda m=m: proj_kk0(m)) for m in (12, 13)]
            if pending_norm is not None:
                fillers.insert(2, pending_norm)
            dns = do_attnv(i, c, cur_exs, fillers, nxt_cl, dn_first=last)
            pending_norm = (lambda c=c, i=i, dns=dns: norm_pair(c, i, dns))
            cur_exs = nxt_exs
        # tail: norm of the last pair (mul on Vector: latency-critical),
        # then the deferred kk=1 halves of m12/m13 and full m14/m15; the
        # last output DMAs are split so their issue overlaps the CASTs
        norm_pair(NSQ - 1, 1, dns, on_vector=True)
        for m in (12, 13):
            ob = osb.tile([P, D], F16, name="ob", tag="ob")
            for nch in range(2):
                nc.tensor.matmul(
                    pf_ps[m][nch],
                    outT[1][:, m * P:(m + 1) * P],
                    wp_t[1][:, nch * SQ:(nch + 1) * SQ],
                    start=False,
                    stop=True,
                )
                with nc.allow_low_precision(reason="partial sums; host sums fp32"):
                    nc.vector.tensor_copy(
                        ob[:, nch * SQ:(nch + 1) * SQ], pf_ps[m][nch])
                nc.sync.dma_start(
                    ap["out"][m * P:(m + 1) * P, nch * SQ:(nch + 1) * SQ],
                    ob[:, nch * SQ:(nch + 1) * SQ],
                )
        for m in (14, 15):
            proj_mtile(m, split_dma=True)


def build_program():
    nc = bacc.Bacc("TRN2", target_bir_lowering=False, debug=False,
                   num_devices=NCORES)
    ap = {}
    for name, shape, dt in (
        ("xln", [P, NSQ, KT, SQ], F16),
        ("wq", [P, KT * CH], F16), ("wk", [P, KT * CH], F16),
        ("wv", [P, KT * HPC * VW], F16), ("wp", [P, 2 * D], F16),
        ("bqk", [P, 4], F32), ("bv", [1, HPC * VW], F16),
        ("tri", [P, P], F16), ("ones1", [1, P], F16),
    ):
        ap[name] = nc.dram_tensor(name, shape, dt, kind="ExternalInput").ap()
    ap["out"] = nc.dram_tensor("out", [S, D], F16, kind="ExternalOutput").ap()

    with tile.TileContext(nc) as tc:
        emit_kernel(nc, tc, ap)
    nc.compile()
    return nc


def make_core_inputs(hidden_states, w_attn, b_attn, w_proj):
    """Host-side sharding: per-core input dicts (core = batch*4 + head_group).

    All tensors are relayouted partition-major so every device DMA is a
    contiguous 128-line transfer (cheap HWDGE descriptor generation)."""
    f16, f32 = np.float16, np.float32
    x = np.asarray(hidden_states, f32)
    w_attn = np.asarray(w_attn, f32)
    b_attn = np.asarray(b_attn, f32)
    w_proj = np.asarray(w_proj, f32)

    tri = (np.arange(P)[:, None] <= np.arange(P)[None, :]).astype(f16)
    ones_row = np.ones((1, P), f16)

    def kmaj(w):  # [D, C] -> [P, KT*C] with w[k*P+p, c] at [p, k*C+c]
        kt = w.shape[0] // P
        return np.ascontiguousarray(
            w.reshape(kt, P, -1).transpose(1, 0, 2).reshape(P, -1)).astype(f16)

    # x[b][c*SQ+s, k*P+p] -> xln[p, c, k, s]
    xlns = [
        np.ascontiguousarray(
            x[b].reshape(NSQ, SQ, KT, P).transpose(3, 0, 2, 1)).astype(f16)
        for b in range(B)
    ]

    in_maps = []
    for core in range(NCORES):
        b, g = core // HPC, core % HPC
        wq = kmaj(w_attn[:, g * CH:(g + 1) * CH])
        wk = kmaj(w_attn[:, D + g * CH:D + (g + 1) * CH])
        wv_full = np.zeros((D, HPC * VW), f32)
        bv = np.zeros((1, HPC * VW), f16)
        for h in range(HPC):
            src = 2 * D + (g * HPC + h) * HD
            wv_full[:, h * VW:h * VW + HD] = w_attn[:, src:src + HD]
            bv[0, h * VW:h * VW + HD] = b_attn[src:src + HD]
            bv[0, h * VW + HD] = 1.0
        wv = kmaj(wv_full)
        bqk = np.zeros((P, 4), f32)
        bqk[:, 0:2] = b_attn[g * CH:(g + 1) * CH].reshape(2, P).T
        bqk[:, 2:4] = b_attn[D + g * CH:D + (g + 1) * CH].reshape(2, P).T
        wp = kmaj(w_proj[g * CH:(g + 1) * CH, :])
        in_maps.append({
            "xln": xlns[b], "wq": wq, "wk": wk, "wv": wv, "wp": wp,
            "bqk": bqk, "bv": bv, "tri": tri, "ones1": ones_row,
        })
    return in_maps


_PROGRAM = None


def kernel(hidden_states, w_attn, b_attn, w_proj, b_proj):
    global _PROGRAM
    if _PROGRAM is None:
        _PROGRAM = build_program()
    in_maps = make_core_inputs(hidden_states, w_attn, b_attn, w_proj)
    res = run_bass_kernel_spmd(_PROGRAM, in_maps, core_ids=list(range(NCORES)))
    out = np.zeros((B, S, D), np.float32)
    for core in range(NCORES):
        out[core // HPC] += res.results[core]["out"].astype(np.float32)
    out += np.asarray(b_proj, np.float32)
    return out


# revision 20
# speedup vs baseline: 1.0187x; 1.0187x over previous
"""Fused causal multi-head attention block on 8 Trainium2 NeuronCores.

Problem (GPT-2 style attention, B=2, S=2048, D=1024, H=16, hd=64):
    qkv = x @ w_attn + b_attn ; split q,k,v ; per-head causal softmax(q k^T / 8) v
    out = attn_out @ w_proj + b_proj

Sharding: data parallel on batch (2) x tensor parallel on heads (4 groups of 4
heads). Core c -> batch c//4, head group c%4. Each core computes a partial
[S, D] output (its heads' slice of w_proj rows); host sums the 4 partials per
batch and adds b_proj.

Per-core kernel layout tricks:
- scores are computed TRANSPOSED (scoresT[key, query]) so the softmax
  denominator falls out of the attn@v matmul by appending a ones-column to v:
  [v | 1]^T @ exp(scoresT) yields the unnormalized output and the per-query
  denominator in one PSUM accumulation.
- matmul inputs are fp16 (full PE rate + fast weight loads); all accumulation
  is fp32 in PSUM. exp(s/8) is in [0, ~13], well inside fp16 range.
- causal masking: fully-masked blocks are skipped via restricted matmul
  widths; diagonal blocks get their exp output multiplied by a 0/1 triangle
  on the (otherwise idle) GpSimd engine, keeping Vector free for psum copies.
- all DRAM tensors are host-relayouted to partition-major contiguous form so
  every DMA issue is a cheap 128-line descriptor; x is loaded per 512-query
  chunk (chunk 0 first) and weights stream in parallel on the Scalar HWDGE
  ring, so real QKV work starts ~8us in with no junk warmup needed.
- emission is chunk-pipelined (QKV chunk c, attention chunk c, projection
  chunk c) so the PE always has dense matmul work while ScalarE runs exp;
  each pair's normalization matmuls are deferred past the next pair's score
  matmuls so the in-order PE never waits on Vector/GpSimd latency.
"""

import sys

sys.path.insert(0, "/opt/trn_rl_repo")

import numpy as np

import concourse.bass as bass
import concourse.mybir as mybir
import concourse.tile as tile
from concourse import bacc
from concourse.bass_utils import run_bass_kernel_spmd

F32 = mybir.dt.float32
F16 = mybir.dt.float16
AFT = mybir.ActivationFunctionType

B, S, D, H, HD = 2, 2048, 1024, 16, 64
NCORES = 8
HPC = 4            # heads per core
CH = HPC * HD      # 256 channels per core
VW = HD + 1        # v width incl. ones column
P = 128
KT = D // P        # 8 contraction tiles over D
SQ = 512           # query/N chunk
NSQ = S // SQ      # 4
NST = S // P       # 16 seq tiles
SCALE = 1.0 / np.sqrt(HD)


def emit_kernel(nc, tc, ap):
    """Emit the per-core program. `ap` is a dict of DRAM APs."""
    with (
        tc.tile_pool(name="const", bufs=1) as cp,
        tc.tile_pool(name="xw", bufs=1) as xw,
        tc.tile_pool(name="act", bufs=1) as acts,
        tc.tile_pool(name="ex", bufs=20) as exp_pool,
        tc.tile_pool(name="dh", bufs=4) as dh_pool,
        tc.tile_pool(name="rc", bufs=2) as rc_pool,
        tc.tile_pool(name="osb", bufs=3) as osb,
        tc.tile_pool(name="psA", bufs=2, space="PSUM") as psA,
        tc.tile_pool(name="psB", bufs=2, space="PSUM") as psB,
        tc.tile_pool(name="psC", bufs=2, space="PSUM") as psC,
    ):
        # ---- input DMAs. Two parallel HWDGE rings: x chunks + small consts
        # on Sync, weights on Scalar. Chunk-0 x and wq are split in halves so
        # the first QKV matmuls can start after ~0.75MB instead of ~1.5MB.
        xts = xw.tile([P, NSQ, KT, SQ], F16, name="xts", tag="xts")
        half = KT // 2
        nc.sync.dma_start(xts[:, 0, 0:half], ap["xln"][:, 0, 0:half])
        nc.sync.dma_start(xts[:, 0, half:KT], ap["xln"][:, 0, half:KT])
        tri = cp.tile([P, P], F16, name="tri", tag="tri")
        nc.sync.dma_start(tri, ap["tri"])
        bqk = cp.tile([P, 4], F32, name="bqk", tag="bqk")
        nc.sync.dma_start(bqk, ap["bqk"])
        bv = cp.tile([1, HPC * VW], F16, name="bv", tag="bv")
        nc.sync.dma_start(bv, ap["bv"])
        ones1 = cp.tile([1, P], F16, name="ones1", tag="ones1")
        nc.sync.dma_start(ones1, ap["ones1"])
        for c in range(1, NSQ):
            nc.sync.dma_start(xts[:, c], ap["xln"][:, c])

        wq = xw.tile([P, KT, CH], F16, name="wq", tag="wq")
        wq_ap = ap["wq"].rearrange("p (k c) -> p k c", k=KT)
        nc.scalar.dma_start(wq[:, 0:half], wq_ap[:, 0:half])
        nc.scalar.dma_start(wq[:, half:KT], wq_ap[:, half:KT])
        wk = xw.tile([P, KT, CH], F16, name="wk", tag="wk")
        nc.scalar.dma_start(wk, ap["wk"].rearrange("p (k c) -> p k c", k=KT))
        wv = xw.tile([P, KT, HPC * VW], F16, name="wv", tag="wv")
        nc.scalar.dma_start(wv, ap["wv"].rearrange("p (k c) -> p k c", k=KT))
        wp = xw.tile([P, 2, D], F16, name="wp", tag="wp")
        nc.scalar.dma_start(wp, ap["wp"].rearrange("p (k c) -> p k c", k=2))

        wq_t = [wq[:, k, :] for k in range(KT)]
        wk_t = [wk[:, k, :] for k in range(KT)]
        wv_t = [wv[:, k, :] for k in range(KT)]
        wp_t = [wp[:, k, :] for k in range(2)]

        # ---- activations living across phases ----
        qT = [acts.tile([P, S], F16, name=f"qT{i}", tag=f"qT{i}") for i in range(2)]
        kTt = [acts.tile([P, S], F16, name=f"kT{i}", tag=f"kT{i}") for i in range(2)]
        vv = acts.tile([P, NST, HPC * VW], F16, name="vv", tag="vv")
        outT = [acts.tile([P, S], F16, name=f"oT{i}", tag=f"oT{i}") for i in range(2)]

        def qkv_qk_group(c, dst, wt, bcol, i):
            ps = psA.tile([P, SQ], F32, name="ps", tag="ps")
            for k in range(KT):
                nc.tensor.matmul(
                    ps,
                    wt[k][:, i * P:(i + 1) * P],
                    xts[:, c, k, :],
                    start=(k == 0),
                    stop=(k == KT - 1),
                )
            with nc.allow_low_precision(reason="fp16 matmul inputs"):
                nc.vector.tensor_scalar_add(
                    dst[i][:, c * SQ:(c + 1) * SQ], ps, bqk[:, bcol + i:bcol + i + 1],
                )

        def qkv_v_group(st):
            # v rows (natural layout + interleaved ones cols)
            ps = psA.tile([P, SQ], F32, name="psv", tag="ps")
            psv = ps[:, 0:HPC * VW]
            for k in range(KT):
                nc.tensor.matmul(
                    psv,
                    xts[:, st // 4, k, (st % 4) * P:(st % 4 + 1) * P],
                    wv_t[k],
                    start=(k == 0),
                    stop=False,
                )
            # += ones_col(seq) x (bv | interleaved 1.0): v-bias + ones col
            nc.tensor.matmul(psv, ones1, bv, start=False, stop=True)
            with nc.allow_low_precision(reason="fp16 matmul inputs"):
                nc.vector.tensor_copy(vv[:, st, :], psv)

        def qkv_groups(c):
            for dst, wt, bcol in ((qT, wq_t, 0), (kTt, wk_t, 2)):
                for i in range(2):
                    yield lambda dst=dst, wt=wt, bcol=bcol, i=i: \
                        qkv_qk_group(c, dst, wt, bcol, i)
            for st in range(4 * c, 4 * c + 4):
                yield lambda st=st: qkv_v_group(st)

        def make_scores(i, c):
            """Score matmul + exp emission closures for pair (c, i), one per
            key tile. Both heads' scores for a key tile land in one 2-bank
            PSUM tile so a single exp instruction covers them. Diagonal key
            tiles go FIRST so their exp->mask chain (mask on the idle GpSimd)
            completes long before attnv consumes them. The closures are
            interleaved into the PREVIOUS pair's attnv phase so ScalarE's exp
            stream never starves at pair boundaries."""
            nkt = 4 * (c + 1)
            kts = list(range(4 * c, nkt)) + list(range(0, 4 * c))
            exs = []

            def emit_kt(kt):
                colo = max(0, kt * P - c * SQ)
                diag = colo > 0 or kt * P == c * SQ
                sc2 = psC.tile([P, 2, SQ], F32, name="sc2", tag="sc")
                for j in range(2):
                    ro = j * 64
                    nc.tensor.matmul(
                        sc2[:, j, colo:SQ],
                        kTt[i][ro:ro + 64, kt * P:(kt + 1) * P],
                        qT[i][ro:ro + 64, c * SQ + colo:(c + 1) * SQ],
                        start=True,
                        stop=True,
                    )
                ex2 = exp_pool.tile([P, 2, SQ], F16, name="ex2", tag="ex")
                nc.scalar.activation(
                    ex2[:, :, colo:SQ], sc2[:, :, colo:SQ], AFT.Exp, scale=SCALE,
                )
                if diag:
                    nc.gpsimd.tensor_mul(
                        ex2[:, :, colo:colo + P],
                        ex2[:, :, colo:colo + P],
                        tri[:, None, :].broadcast_to([P, 2, P]),
                    )
                exs.append((ex2, kt, colo))

            return [(lambda kt=kt: emit_kt(kt)) for kt in kts], exs

        def do_attnv(i, c, exs, fillers, next_scores, dn_first=False):
            """attnv accumulation for pair (c, i), interleaved with the next
            pair's score/exp emissions (to keep ScalarE saturated) and dense
            PE fillers (qkv/proj work)."""
            nkt = 4 * (c + 1)
            fillers = list(fillers)
            next_scores = list(next_scores)
            nf, ns = len(fillers), len(next_scores)
            accs = [psB.tile([VW, SQ], F32, name="acc", tag="acc")
                    for _ in range(2)]
            # pre-work before the first attnv matmul: covers the PSUM
            # acc-slot WAR on the previous pair's psum->sbuf copies and
            # jump-starts the next pair's exp stream
            for pre in range(2):
                if fillers:
                    fillers.pop(0)()
                if next_scores:
                    next_scores.pop(0)()
            for idx, (ex2, kt, colo) in enumerate(exs):
                for j in range(2):
                    h = 2 * i + j
                    nc.tensor.matmul(
                        accs[j][:, colo:SQ],
                        vv[:, kt, h * VW:(h + 1) * VW],
                        ex2[:, j, colo:SQ],
                        start=(idx == 0),
                        stop=(idx == nkt - 1),
                    )
                while next_scores and \
                        (ns - len(next_scores)) < ns * (idx + 1) // nkt:
                    next_scores.pop(0)()
                while fillers and len(fillers) > nf * (nkt - 1 - idx) // nkt:
                    fillers.pop(0)()
            # psum->sbuf copies; outT first so the next pair's attnv can
            # reuse the acc psum slots ASAP. At the tail (dn_first) the norm
            # chain is the critical path: dns go first on Vector and the outT
            # copies move to the now-idle ScalarE so Vector's serial chain is
            # just dn -> reciprocal -> mul.
            dns = []
            if dn_first:
                for j in range(2):
                    dn = dh_pool.tile([1, SQ], F16, name="dn", tag="dn")
                    with nc.allow_low_precision(reason="fp16 matmul inputs"):
                        nc.vector.tensor_copy(dn, accs[j][64:65, :])
                    dns.append(dn)
            for j in range(2):
                with nc.allow_low_precision(reason="fp16 matmul inputs"):
                    if dn_first:
                        nc.scalar.activation(
                            outT[i][j * 64:j * 64 + 64, c * SQ:(c + 1) * SQ],
                            accs[j][0:64, :], AFT.Copy,
                        )
                    else:
                        nc.vector.tensor_copy(
                            outT[i][j * 64:j * 64 + 64, c * SQ:(c + 1) * SQ],
                            accs[j][0:64, :],
                        )
            if not dn_first:
                for j in range(2):
                    dn = dh_pool.tile([1, SQ], F16, name="dn", tag="dn")
                    with nc.allow_low_precision(reason="fp16 matmul inputs"):
                        nc.vector.tensor_copy(dn, accs[j][64:65, :])
                    dns.append(dn)
            return dns

        def norm_pair(c, i, dns, on_vector=False, db_pool=None):
            # outT *= 1/denominator: broadcast denoms via K=1 matmuls, one
            # 128-lane fast reciprocal, one fp16 multiply (on GpSimd
            # mid-kernel; on Vector at the tail where latency matters)
            db = (db_pool or psA).tile([P, SQ], F32, name="db", tag="acc" if db_pool else "ps")
            nc.tensor.matmul(
                db[0:64, :], ones1[:, 0:64], dns[0],
                start=True, stop=True,
            )
            nc.tensor.matmul(
                db[64:P, :], ones1[:, 0:64], dns[1],
                start=True, stop=True,
            )
            rc32 = rc_pool.tile([P, SQ], F32, name="rc32", tag="rc32")
            nc.vector.reciprocal_approx_fast(rc32, db)
            eng = nc.vector if on_vector else nc.gpsimd
            with nc.allow_low_precision(reason="fp16 matmul inputs"):
                eng.tensor_mul(
                    outT[i][:, c * SQ:(c + 1) * SQ],
                    outT[i][:, c * SQ:(c + 1) * SQ],
                    rc32,
                )

        def proj_mtile(m, split_dma=False):
            ob = osb.tile([P, D], F16, name="ob", tag="ob")
            for nch in range(2):
                ps = psA.tile([P, SQ], F32, name="ps", tag="ps")
                for kk in range(2):
                    nc.tensor.matmul(
                        ps,
                        outT[kk][:, m * P:(m + 1) * P],
                        wp_t[kk][:, nch * SQ:(nch + 1) * SQ],
                        start=(kk == 0),
                        stop=(kk == 1),
                    )
                with nc.allow_low_precision(reason="partial sums; host sums fp32"):
                    nc.vector.tensor_copy(ob[:, nch * SQ:(nch + 1) * SQ], ps)
                if split_dma:
                    nc.sync.dma_start(
                        ap["out"][m * P:(m + 1) * P, nch * SQ:(nch + 1) * SQ],
                        ob[:, nch * SQ:(nch + 1) * SQ],
                    )
            if not split_dma:
                nc.sync.dma_start(ap["out"][m * P:(m + 1) * P, :], ob)

        pf_ps = {}

        def proj_kk0(m):
            # kk=0 contraction half of a tail projection tile, parked in
            # psC (free once the last exps drain) until norm(3,1) releases
            # outT[1] for the kk=1 half
            ps2 = psC.tile([P, 2, SQ], F32, name="pf", tag="sc")
            pf_ps[m] = [ps2[:, 0, :], ps2[:, 1, :]]
            for nch in range(2):
                nc.tensor.matmul(
                    pf_ps[m][nch],
                    outT[0][:, m * P:(m + 1) * P],
                    wp_t[0][:, nch * SQ:(nch + 1) * SQ],
                    start=True,
                    stop=False,
                )

        # ---- chunk-pipelined main body ----
        # chunk 0 QKV runs as soon as chunk-0 x + weights stream in; the
        # PE's HAM cold-start window is spent on this real work.
        for g in qkv_groups(0):
            g()
        # Pair p's scores/exps are interleaved into pair p-1's attnv phase
        # so both the PE and ScalarE stay saturated; qkv chunk c+1 and the
        # ready projection tiles serve as dense PE filler inside the
        # exp-paced attnv windows. Each pair's norm is deferred two slots
        # into the NEXT pair's filler stream (past the Vector dn-copies).
        pairs = [(c, i) for c in range(NSQ) for i in range(2)]
        cl0, cur_exs = make_scores(0, 0)
        for cl in cl0:
            cl()
        pending_norm = None
        for pidx, (c, i) in enumerate(pairs):
            last = pidx == len(pairs) - 1
            if not last:
                nxt_c, nxt_i = pairs[pidx + 1]
                nxt_cl, nxt_exs = make_scores(nxt_i, nxt_c)
            else:
                nxt_cl, nxt_exs = [], None
            fillers = []
            if c + 1 < NSQ:
                nxt_qkv = list(qkv_groups(c + 1))
                fillers += nxt_qkv[4 * i:4 * i + 4]
            if c == NSQ - 2 and i == 1:
                fillers += [lambda: proj_mtile(0), lambda: proj_mtile(1)]
            if c == NSQ - 1:
                ms = range(2, 8) if i == 0 else range(8, 12)
                fillers += [(lambda m=m: proj_mtile(m)) for m in ms]
            if pending_norm is not None:
                fillers.insert(2, pending_norm)
            dns = do_attnv(i, c, cur_exs, fillers, nxt_cl, dn_first=last)
            pending_norm = (lambda c=c, i=i, dns=dns: norm_pair(c, i, dns))
            cur_exs = nxt_exs
        # tail. While Vector runs the dn->reciprocal->mul chain of norm(3,1)
        # (outT copies went to ScalarE), the PE prefetches every kk=0
        # contraction half it has PSUM for: m12/m13 into the drained psC,
        # m14 into psA; db goes to psB (its acc-slot WAR equals its real dn
        # dependency). After the mul, only kk=1 halves + m15 remain.
        proj_kk0(12)
        proj_kk0(13)
        pf_ps[14] = [psA.tile([P, SQ], F32, name="ps", tag="ps")
                     for _ in range(2)]
        for nch in range(2):
            nc.tensor.matmul(
                pf_ps[14][nch],
                outT[0][:, 14 * P:15 * P],
                wp_t[0][:, nch * SQ:(nch + 1) * SQ],
                start=True, stop=False,
            )
        norm_pair(NSQ - 1, 1, dns, on_vector=True, db_pool=psB)
        for m in (12, 13, 14):
            ob = osb.tile([P, D], F16, name="ob", tag="ob")
            for nch in range(2):
                nc.tensor.matmul(
                    pf_ps[m][nch],
                    outT[1][:, m * P:(m + 1) * P],
                    wp_t[1][:, nch * SQ:(nch + 1) * SQ],
                    start=False,
                    stop=True,
                )
                with nc.allow_low_precision(reason="partial sums; host sums fp32"):
                    nc.vector.tensor_copy(
                        ob[:, nch * SQ:(nch + 1) * SQ], pf_ps[m][nch])
                nc.sync.dma_start(
                    ap["out"][m * P:(m + 1) * P, nch * SQ:(nch + 1) * SQ],
                    ob[:, nch * SQ:(nch + 1) * SQ],
                )
        proj_mtile(15, split_dma=True)


def build_program():
    nc = bacc.Bacc("TRN2", target_bir_lowering=False, debug=False,
                   num_devices=NCORES)
    ap = {}
    for name, shape, dt in (
        ("xln", [P, NSQ, KT, SQ], F16),
        ("wq", [P, KT * CH], F16), ("wk", [P, KT * CH], F16),
        ("wv", [P, KT * HPC * VW], F16), ("wp", [P, 2 * D], F16),
        ("bqk", [P, 4], F32), ("bv", [1, HPC * VW], F16),
        ("tri", [P, P], F16), ("ones1", [1, P], F16),
    ):
        ap[name] = nc.dram_tensor(name, shape, dt, kind="ExternalInput").ap()
    ap["out"] = nc.dram_tensor("out", [S, D], F16, kind="ExternalOutput").ap()

    with tile.TileContext(nc) as tc:
        emit_kernel(nc, tc, ap)
    nc.compile()
    return nc


def make_core_inputs(hidden_states, w_attn, b_attn, w_proj):
    """Host-side sharding: per-core input dicts (core = batch*4 + head_group).

    All tensors are relayouted partition-major so every device DMA is a
    contiguous 128-line transfer (cheap HWDGE descriptor generation)."""
    f16, f32 = np.float16, np.float32
    x = np.asarray(hidden_states, f32)
    w_attn = np.asarray(w_attn, f32)
    b_attn = np.asarray(b_attn, f32)
    w_proj = np.asarray(w_proj, f32)

    tri = (np.arange(P)[:, None] <= np.arange(P)[None, :]).astype(f16)
    ones_row = np.ones((1, P), f16)

    def kmaj(w):  # [D, C] -> [P, KT*C] with w[k*P+p, c] at [p, k*C+c]
        kt = w.shape[0] // P
        return np.ascontiguousarray(
            w.reshape(kt, P, -1).transpose(1, 0, 2).reshape(P, -1)).astype(f16)

    # x[b][c*SQ+s, k*P+p] -> xln[p, c, k, s]
    xlns = [
        np.ascontiguousarray(
            x[b].reshape(NSQ, SQ, KT, P).transpose(3, 0, 2, 1)).astype(f16)
        for b in range(B)
    ]

    in_maps = []
    for core in range(NCORES):
        b, g = core // HPC, core % HPC
        wq = kmaj(w_attn[:, g * CH:(g + 1) * CH])
        wk = kmaj(w_attn[:, D + g * CH:D + (g + 1) * CH])
        wv_full = np.zeros((D, HPC * VW), f32)
        bv = np.zeros((1, HPC * VW), f16)
        for h in range(HPC):
            src = 2 * D + (g * HPC + h) * HD
            wv_full[:, h * VW:h * VW + HD] = w_attn[:, src:src + HD]
            bv[0, h * VW:h * VW + HD] = b_attn[src:src + HD]
            bv[0, h * VW + HD] = 1.0
        wv = kmaj(wv_full)
        bqk = np.zeros((P, 4), f32)
        bqk[:, 0:2] = b_attn[g * CH:(g + 1) * CH].reshape(2, P).T
        bqk[:, 2:4] = b_attn[D + g * CH:D + (g + 1) * CH].reshape(2, P).T
        wp = kmaj(w_proj[g * CH:(g + 1) * CH, :])
        in_maps.append({
            "xln": xlns[b], "wq": wq, "wk": wk, "wv": wv, "wp": wp,
            "bqk": bqk, "bv": bv, "tri": tri, "ones1": ones_row,
        })
    return in_maps


_PROGRAM = None


def kernel(hidden_states, w_attn, b_attn, w_proj, b_proj):
    global _PROGRAM
    if _PROGRAM is None:
        _PROGRAM = build_program()
    in_maps = make_core_inputs(hidden_states, w_attn, b_attn, w_proj)
    res = run_bass_kernel_spmd(_PROGRAM, in_maps, core_ids=list(range(NCORES)))
    out = np.zeros((B, S, D), np.float32)
    for core in range(NCORES):
        out[core // HPC] += res.results[core]["out"].astype(np.float32)
    out += np.asarray(b_proj, np.float32)
    return out


# revision 23
# speedup vs baseline: 1.0224x; 1.0036x over previous
"""Fused causal multi-head attention block on 8 Trainium2 NeuronCores.

Problem (GPT-2 style attention, B=2, S=2048, D=1024, H=16, hd=64):
    qkv = x @ w_attn + b_attn ; split q,k,v ; per-head causal softmax(q k^T / 8) v
    out = attn_out @ w_proj + b_proj

Sharding: data parallel on batch (2) x tensor parallel on heads (4 groups of 4
heads). Core c -> batch c//4, head group c%4. Each core computes a partial
[S, D] output (its heads' slice of w_proj rows); host sums the 4 partials per
batch and adds b_proj.

Per-core kernel layout tricks:
- scores are computed TRANSPOSED (scoresT[key, query]) so the softmax
  denominator falls out of the attn@v matmul by appending a ones-column to v:
  [v | 1]^T @ exp(scoresT) yields the unnormalized output and the per-query
  denominator in one PSUM accumulation.
- matmul inputs are fp16 (full PE rate + fast weight loads); all accumulation
  is fp32 in PSUM. exp(s/8) is in [0, ~13], well inside fp16 range.
- causal masking: fully-masked blocks are skipped via restricted matmul
  widths; diagonal blocks get their exp output multiplied by a 0/1 triangle
  on the (otherwise idle) GpSimd engine, keeping Vector free for psum copies.
- all DRAM tensors are host-relayouted to partition-major contiguous form so
  every DMA issue is a cheap 128-line descriptor; x is loaded per 512-query
  chunk (chunk 0 first) and weights stream in parallel on the Scalar HWDGE
  ring, so real QKV work starts ~8us in with no junk warmup needed.
- emission is chunk-pipelined (QKV chunk c, attention chunk c, projection
  chunk c) so the PE always has dense matmul work while ScalarE runs exp;
  each pair's normalization matmuls are deferred past the next pair's score
  matmuls so the in-order PE never waits on Vector/GpSimd latency.
"""

import sys

sys.path.insert(0, "/opt/trn_rl_repo")

import numpy as np

import concourse.bass as bass
import concourse.mybir as mybir
import concourse.tile as tile
from concourse import bacc
from concourse.bass_utils import run_bass_kernel_spmd

F32 = mybir.dt.float32
F16 = mybir.dt.float16
AFT = mybir.ActivationFunctionType

B, S, D, H, HD = 2, 2048, 1024, 16, 64
NCORES = 8
HPC = 4            # heads per core
CH = HPC * HD      # 256 channels per core
VW = HD + 1        # v width incl. ones column
P = 128
KT = D // P        # 8 contraction tiles over D
SQ = 512           # query/N chunk
NSQ = S // SQ      # 4
NST = S // P       # 16 seq tiles
SCALE = 1.0 / np.sqrt(HD)


def emit_kernel(nc, tc, ap):
    """Emit the per-core program. `ap` is a dict of DRAM APs."""
    with (
        tc.tile_pool(name="const", bufs=1) as cp,
        tc.tile_pool(name="xw", bufs=1) as xw,
        tc.tile_pool(name="act", bufs=1) as acts,
        tc.tile_pool(name="ex", bufs=32) as exp_pool,
        tc.tile_pool(name="dh", bufs=4) as dh_pool,
        tc.tile_pool(name="rc", bufs=2) as rc_pool,
        tc.tile_pool(name="osb", bufs=3) as osb,
        tc.tile_pool(name="psA", bufs=2, space="PSUM") as psA,
        tc.tile_pool(name="psB", bufs=2, space="PSUM") as psB,
        tc.tile_pool(name="psC", bufs=2, space="PSUM") as psC,
    ):
        # ---- input DMAs. Two parallel HWDGE rings: x chunks + small consts
        # on Sync, weights on Scalar. Chunk-0 x and wq are split in halves so
        # the first QKV matmuls can start after ~0.75MB instead of ~1.5MB.
        xts = xw.tile([P, NSQ, KT, SQ], F16, name="xts", tag="xts")
        half = KT // 2
        nc.sync.dma_start(xts[:, 0, 0:half], ap["xln"][:, 0, 0:half])
        nc.sync.dma_start(xts[:, 0, half:KT], ap["xln"][:, 0, half:KT])
        tri = cp.tile([P, P], F16, name="tri", tag="tri")
        nc.sync.dma_start(tri, ap["tri"])
        bqk = cp.tile([P, 4], F32, name="bqk", tag="bqk")
        nc.sync.dma_start(bqk, ap["bqk"])
        bv = cp.tile([1, HPC * VW], F16, name="bv", tag="bv")
        nc.sync.dma_start(bv, ap["bv"])
        ones1 = cp.tile([1, P], F16, name="ones1", tag="ones1")
        nc.sync.dma_start(ones1, ap["ones1"])
        for c in range(1, NSQ):
            nc.sync.dma_start(xts[:, c], ap["xln"][:, c])

        wq = xw.tile([P, KT, CH], F16, name="wq", tag="wq")
        wq_ap = ap["wq"].rearrange("p (k c) -> p k c", k=KT)
        nc.scalar.dma_start(wq[:, 0:half], wq_ap[:, 0:half])
        nc.scalar.dma_start(wq[:, half:KT], wq_ap[:, half:KT])
        wk = xw.tile([P, KT, CH], F16, name="wk", tag="wk")
        nc.scalar.dma_start(wk, ap["wk"].rearrange("p (k c) -> p k c", k=KT))
        wv = xw.tile([P, KT, HPC * VW], F16, name="wv", tag="wv")
        nc.scalar.dma_start(wv, ap["wv"].rearrange("p (k c) -> p k c", k=KT))
        wp = xw.tile([P, 2, D], F16, name="wp", tag="wp")
        nc.scalar.dma_start(wp, ap["wp"].rearrange("p (k c) -> p k c", k=2))

        wq_t = [wq[:, k, :] for k in range(KT)]
        wk_t = [wk[:, k, :] for k in range(KT)]
        wv_t = [wv[:, k, :] for k in range(KT)]
        wp_t = [wp[:, k, :] for k in range(2)]

        # ---- activations living across phases ----
        qT = [acts.tile([P, S], F16, name=f"qT{i}", tag=f"qT{i}") for i in range(2)]
        kTt = [acts.tile([P, S], F16, name=f"kT{i}", tag=f"kT{i}") for i in range(2)]
        vv = acts.tile([P, NST, HPC * VW], F16, name="vv", tag="vv")
        outT = [acts.tile([P, S], F16, name=f"oT{i}", tag=f"oT{i}") for i in range(2)]

        def qkv_qk_group(c, dst, wt, bcol, i):
            ps = psA.tile([P, SQ], F32, name="ps", tag="ps")
            for k in range(KT):
                nc.tensor.matmul(
                    ps,
                    wt[k][:, i * P:(i + 1) * P],
                    xts[:, c, k, :],
                    start=(k == 0),
                    stop=(k == KT - 1),
                )
            with nc.allow_low_precision(reason="fp16 matmul inputs"):
                nc.vector.tensor_scalar_add(
                    dst[i][:, c * SQ:(c + 1) * SQ], ps, bqk[:, bcol + i:bcol + i + 1],
                )

        def qkv_v_group(st):
            # v rows (natural layout + interleaved ones cols)
            ps = psA.tile([P, SQ], F32, name="psv", tag="ps")
            psv = ps[:, 0:HPC * VW]
            for k in range(KT):
                nc.tensor.matmul(
                    psv,
                    xts[:, st // 4, k, (st % 4) * P:(st % 4 + 1) * P],
                    wv_t[k],
                    start=(k == 0),
                    stop=False,
                )
            # += ones_col(seq) x (bv | interleaved 1.0): v-bias + ones col
            nc.tensor.matmul(psv, ones1, bv, start=False, stop=True)
            with nc.allow_low_precision(reason="fp16 matmul inputs"):
                nc.vector.tensor_copy(vv[:, st, :], psv)

        def qkv_groups(c):
            for dst, wt, bcol in ((qT, wq_t, 0), (kTt, wk_t, 2)):
                for i in range(2):
                    yield lambda dst=dst, wt=wt, bcol=bcol, i=i: \
                        qkv_qk_group(c, dst, wt, bcol, i)
            for st in range(4 * c, 4 * c + 4):
                yield lambda st=st: qkv_v_group(st)

        def make_scores(i, c):
            """Score matmul + exp emission closures for pair (c, i), one per
            key tile. Both heads' scores for a key tile land in one 2-bank
            PSUM tile so a single exp instruction covers them. Diagonal key
            tiles go FIRST so their exp->mask chain (mask on the idle GpSimd)
            completes long before attnv consumes them. The closures are
            interleaved into the PREVIOUS pair's attnv phase so ScalarE's exp
            stream never starves at pair boundaries."""
            nkt = 4 * (c + 1)
            kts = list(range(4 * c, nkt)) + list(range(0, 4 * c))
            exs = []

            def emit_kt(kt):
                colo = max(0, kt * P - c * SQ)
                diag = colo > 0 or kt * P == c * SQ
                sc2 = psC.tile([P, 2, SQ], F32, name="sc2", tag="sc")
                for j in range(2):
                    ro = j * 64
                    nc.tensor.matmul(
                        sc2[:, j, colo:SQ],
                        kTt[i][ro:ro + 64, kt * P:(kt + 1) * P],
                        qT[i][ro:ro + 64, c * SQ + colo:(c + 1) * SQ],
                        start=True,
                        stop=True,
                    )
                ex2 = exp_pool.tile([P, 2, SQ], F16, name="ex2", tag="ex")
                nc.scalar.activation(
                    ex2[:, :, colo:SQ], sc2[:, :, colo:SQ], AFT.Exp, scale=SCALE,
                )
                if diag:
                    nc.gpsimd.tensor_mul(
                        ex2[:, :, colo:colo + P],
                        ex2[:, :, colo:colo + P],
                        tri[:, None, :].broadcast_to([P, 2, P]),
                    )
                exs.append((ex2, kt, colo))

            return [(lambda kt=kt: emit_kt(kt)) for kt in kts], exs

        def do_attnv(i, c, exs, fillers, next_scores, dn_first=False):
            """attnv accumulation for pair (c, i), interleaved with the next
            pair's score/exp emissions (to keep ScalarE saturated) and dense
            PE fillers (qkv/proj work)."""
            nkt = 4 * (c + 1)
            fillers = list(fillers)
            next_scores = list(next_scores)
            nf, ns = len(fillers), len(next_scores)
            accs = [psB.tile([VW, SQ], F32, name="acc", tag="acc")
                    for _ in range(2)]
            # pre-work before the first attnv matmul: covers the PSUM
            # acc-slot WAR on the previous pair's psum->sbuf copies and
            # jump-starts the next pair's exp stream
            for pre in range(2):
                if fillers:
                    fillers.pop(0)()
                if next_scores:
                    next_scores.pop(0)()
            for idx, (ex2, kt, colo) in enumerate(exs):
                for j in range(2):
                    h = 2 * i + j
                    nc.tensor.matmul(
                        accs[j][:, colo:SQ],
                        vv[:, kt, h * VW:(h + 1) * VW],
                        ex2[:, j, colo:SQ],
                        start=(idx == 0),
                        stop=(idx == nkt - 1),
                    )
                while next_scores and \
                        (ns - len(next_scores)) < ns * (idx + 1) // nkt:
                    next_scores.pop(0)()
                while fillers and len(fillers) > nf * (nkt - 1 - idx) // nkt:
                    fillers.pop(0)()
            # psum->sbuf copies; outT first so the next pair's attnv can
            # reuse the acc psum slots ASAP. At the tail (dn_first) the norm
            # chain is the critical path: dns go first on Vector and the outT
            # copies move to the now-idle ScalarE so Vector's serial chain is
            # just dn -> reciprocal -> mul.
            dns = []
            if dn_first:
                for j in range(2):
                    dn = dh_pool.tile([1, SQ], F16, name="dn", tag="dn")
                    with nc.allow_low_precision(reason="fp16 matmul inputs"):
                        nc.vector.tensor_copy(dn, accs[j][64:65, :])
                    dns.append(dn)
            for j in range(2):
                with nc.allow_low_precision(reason="fp16 matmul inputs"):
                    if dn_first:
                        nc.scalar.activation(
                            outT[i][j * 64:j * 64 + 64, c * SQ:(c + 1) * SQ],
                            accs[j][0:64, :], AFT.Copy,
                        )
                    else:
                        nc.vector.tensor_copy(
                            outT[i][j * 64:j * 64 + 64, c * SQ:(c + 1) * SQ],
                            accs[j][0:64, :],
                        )
            if not dn_first:
                for j in range(2):
                    dn = dh_pool.tile([1, SQ], F16, name="dn", tag="dn")
                    with nc.allow_low_precision(reason="fp16 matmul inputs"):
                        nc.vector.tensor_copy(dn, accs[j][64:65, :])
                    dns.append(dn)
            return dns

        def norm_pair(c, i, dns, on_vector=False, db_pool=None):
            # outT *= 1/denominator: broadcast denoms via K=1 matmuls, one
            # 128-lane fast reciprocal, one fp16 multiply (on GpSimd
            # mid-kernel; on Vector at the tail where latency matters)
            db = (db_pool or psA).tile([P, SQ], F32, name="db", tag="acc" if db_pool else "ps")
            nc.tensor.matmul(
                db[0:64, :], ones1[:, 0:64], dns[0],
                start=True, stop=True,
            )
            nc.tensor.matmul(
                db[64:P, :], ones1[:, 0:64], dns[1],
                start=True, stop=True,
            )
            rc32 = rc_pool.tile([P, SQ], F32, name="rc32", tag="rc32")
            nc.vector.reciprocal_approx_fast(rc32, db)
            eng = nc.vector if on_vector else nc.gpsimd
            with nc.allow_low_precision(reason="fp16 matmul inputs"):
                eng.tensor_mul(
                    outT[i][:, c * SQ:(c + 1) * SQ],
                    outT[i][:, c * SQ:(c + 1) * SQ],
                    rc32,
                )

        def proj_mtile(m, split_dma=False):
            ob = osb.tile([P, D], F16, name="ob", tag="ob")
            for nch in range(2):
                ps = psA.tile([P, SQ], F32, name="ps", tag="ps")
                for kk in range(2):
                    nc.tensor.matmul(
                        ps,
                        outT[kk][:, m * P:(m + 1) * P],
                        wp_t[kk][:, nch * SQ:(nch + 1) * SQ],
                        start=(kk == 0),
                        stop=(kk == 1),
                    )
                with nc.allow_low_precision(reason="partial sums; host sums fp32"):
                    nc.vector.tensor_copy(ob[:, nch * SQ:(nch + 1) * SQ], ps)
                if split_dma:
                    nc.sync.dma_start(
                        ap["out"][m * P:(m + 1) * P, nch * SQ:(nch + 1) * SQ],
                        ob[:, nch * SQ:(nch + 1) * SQ],
                    )
            if not split_dma:
                nc.sync.dma_start(ap["out"][m * P:(m + 1) * P, :], ob)

        pf_ps = {}

        def proj_kk0(m):
            # kk=0 contraction half of a tail projection tile, parked in
            # psC (free once the last exps drain) until norm(3,1) releases
            # outT[1] for the kk=1 half
            ps2 = psC.tile([P, 2, SQ], F32, name="pf", tag="sc")
            pf_ps[m] = [ps2[:, 0, :], ps2[:, 1, :]]
            for nch in range(2):
                nc.tensor.matmul(
                    pf_ps[m][nch],
                    outT[0][:, m * P:(m + 1) * P],
                    wp_t[0][:, nch * SQ:(nch + 1) * SQ],
                    start=True,
                    stop=False,
                )

        # ---- chunk-pipelined main body ----
        # chunk 0 QKV runs as soon as chunk-0 x + weights stream in; the
        # PE's HAM cold-start window is spent on this real work.
        for g in qkv_groups(0):
            g()
        # Pair p's scores/exps are interleaved into pair p-1's attnv phase
        # so both the PE and ScalarE stay saturated; qkv chunk c+1 and the
        # ready projection tiles serve as dense PE filler inside the
        # exp-paced attnv windows. Each pair's norm is deferred two slots
        # into the NEXT pair's filler stream (past the Vector dn-copies).
        pairs = [(c, i) for c in range(NSQ) for i in range(2)]
        cl0, cur_exs = make_scores(0, 0)
        for cl in cl0:
            cl()
        pending_norm = None
        for pidx, (c, i) in enumerate(pairs):
            last = pidx == len(pairs) - 1
            if not last:
                nxt_c, nxt_i = pairs[pidx + 1]
                nxt_cl, nxt_exs = make_scores(nxt_i, nxt_c)
            else:
                nxt_cl, nxt_exs = [], None
            fillers = []
            if c + 1 < NSQ:
                nxt_qkv = list(qkv_groups(c + 1))
                fillers += nxt_qkv[4 * i:4 * i + 4]
            if c == NSQ - 2 and i == 1:
                fillers += [lambda: proj_mtile(0)]
            if c == NSQ - 1:
                ms = range(1, 8) if i == 0 else range(8, 12)
                fillers += [(lambda m=m: proj_mtile(m)) for m in ms]
            if pending_norm is not None:
                fillers.insert(2, pending_norm)
            dns = do_attnv(i, c, cur_exs, fillers, nxt_cl, dn_first=last)
            pending_norm = (lambda c=c, i=i, dns=dns: norm_pair(c, i, dns))
            cur_exs = nxt_exs
        # tail. While Vector runs the dn->reciprocal->mul chain of norm(3,1)
        # (outT copies went to ScalarE), the PE prefetches ALL four tail
        # tiles' kk=0 contraction halves: m12/m13 into the drained psC, m14
        # into psA, m15 into psB behind db (each WAR matches an already-
        # emitted consumer, so no deadlock). After the mul, only the kk=1
        # halves remain, then CAST + split output DMAs.
        proj_kk0(12)
        proj_kk0(13)
        pf_ps[14] = [psA.tile([P, SQ], F32, name="ps", tag="ps")
                     for _ in range(2)]
        for nch in range(2):
            nc.tensor.matmul(
                pf_ps[14][nch],
                outT[0][:, 14 * P:15 * P],
                wp_t[0][:, nch * SQ:(nch + 1) * SQ],
                start=True, stop=False,
            )
        norm_pair(NSQ - 1, 1, dns, on_vector=True, db_pool=psB)
        pf_ps[15] = [psB.tile([P, SQ], F32, name="db", tag="acc")
                     for _ in range(2)]
        for nch in range(2):
            nc.tensor.matmul(
                pf_ps[15][nch],
                outT[0][:, 15 * P:16 * P],
                wp_t[0][:, nch * SQ:(nch + 1) * SQ],
                start=True, stop=False,
            )
        for m in (12, 13, 14, 15):
            ob = osb.tile([P, D], F16, name="ob", tag="ob")
            for nch in range(2):
                nc.tensor.matmul(
                    pf_ps[m][nch],
                    outT[1][:, m * P:(m + 1) * P],
                    wp_t[1][:, nch * SQ:(nch + 1) * SQ],
                    start=False,
                    stop=True,
                )
                with nc.allow_low_precision(reason="partial sums; host sums fp32"):
                    nc.vector.tensor_copy(
                        ob[:, nch * SQ:(nch + 1) * SQ], pf_ps[m][nch])
                nc.sync.dma_start(
                    ap["out"][m * P:(m + 1) * P, nch * SQ:(nch + 1) * SQ],
                    ob[:, nch * SQ:(nch + 1) * SQ],
                )


def build_program():
    nc = bacc.Bacc("TRN2", target_bir_lowering=False, debug=False,
                   num_devices=NCORES)
    ap = {}
    for name, shape, dt in (
        ("xln", [P, NSQ, KT, SQ], F16),
        ("wq", [P, KT * CH], F16), ("wk", [P, KT * CH], F16),
        ("wv", [P, KT * HPC * VW], F16), ("wp", [P, 2 * D], F16),
        ("bqk", [P, 4], F32), ("bv", [1, HPC * VW], F16),
        ("tri", [P, P], F16), ("ones1", [1, P], F16),
    ):
        ap[name] = nc.dram_tensor(name, shape, dt, kind="ExternalInput").ap()
    ap["out"] = nc.dram_tensor("out", [S, D], F16, kind="ExternalOutput").ap()

    with tile.TileContext(nc) as tc:
        emit_kernel(nc, tc, ap)
    nc.compile()
    return nc


def make_core_inputs(hidden_states, w_attn, b_attn, w_proj):
    """Host-side sharding: per-core input dicts (core = batch*4 + head_group).

    All tensors are relayouted partition-major so every device DMA is a
    contiguous 128-line transfer (cheap HWDGE descriptor generation)."""
    f16, f32 = np.float16, np.float32
    x = np.asarray(hidden_states, f32)
    w_attn = np.asarray(w_attn, f32)
    b_attn = np.asarray(b_attn, f32)
    w_proj = np.asarray(w_proj, f32)

    tri = (np.arange(P)[:, None] <= np.arange(P)[None, :]).astype(f16)
    ones_row = np.ones((1, P), f16)

    def kmaj(w):  # [D, C] -> [P, KT*C] with w[k*P+p, c] at [p, k*C+c]
        kt = w.shape[0] // P
        return np.ascontiguousarray(
            w.reshape(kt, P, -1).transpose(1, 0, 2).reshape(P, -1)).astype(f16)

    # x[b][c*SQ+s, k*P+p] -> xln[p, c, k, s]
    xlns = [
        np.ascontiguousarray(
            x[b].reshape(NSQ, SQ, KT, P).transpose(3, 0, 2, 1)).astype(f16)
        for b in range(B)
    ]

    in_maps = []
    for core in range(NCORES):
        b, g = core // HPC, core % HPC
        wq = kmaj(w_attn[:, g * CH:(g + 1) * CH])
        wk = kmaj(w_attn[:, D + g * CH:D + (g + 1) * CH])
        wv_full = np.zeros((D, HPC * VW), f32)
        bv = np.zeros((1, HPC * VW), f16)
        for h in range(HPC):
            src = 2 * D + (g * HPC + h) * HD
            wv_full[:, h * VW:h * VW + HD] = w_attn[:, src:src + HD]
            bv[0, h * VW:h * VW + HD] = b_attn[src:src + HD]
            bv[0, h * VW + HD] = 1.0
        wv = kmaj(wv_full)
        bqk = np.zeros((P, 4), f32)
        bqk[:, 0:2] = b_attn[g * CH:(g + 1) * CH].reshape(2, P).T
        bqk[:, 2:4] = b_attn[D + g * CH:D + (g + 1) * CH].reshape(2, P).T
        wp = kmaj(w_proj[g * CH:(g + 1) * CH, :])
        in_maps.append({
            "xln": xlns[b], "wq": wq, "wk": wk, "wv": wv, "wp": wp,
            "bqk": bqk, "bv": bv, "tri": tri, "ones1": ones_row,
        })
    return in_maps


_PROGRAM = None


def kernel(hidden_states, w_attn, b_attn, w_proj, b_proj):
    global _PROGRAM
    if _PROGRAM is None:
        _PROGRAM = build_program()
    in_maps = make_core_inputs(hidden_states, w_attn, b_attn, w_proj)
    res = run_bass_kernel_spmd(_PROGRAM, in_maps, core_ids=list(range(NCORES)))
    out = np.zeros((B, S, D), np.float32)
    for core in range(NCORES):
        out[core // HPC] += res.results[core]["out"].astype(np.float32)
    out += np.asarray(b_proj, np.float32)
    return out
